# revision 33
# baseline (speedup 1.0000x reference)
"""Trainium2 Bass kernel for nn_Blur2: depthwise 4x4 blur (upfirdn2d-style,
pad=(2,1), unit stride) over input [8, 128, 256, 256] f32.

Strategy: pure data parallel over the 1024 independent (n, c) planes --
128 planes per NeuronCore. Within a plane the 2D 16-tap conv runs on the
tensor engine as banded matmuls: the H-direction conv is the contraction
(banded Toeplitz fp16 weights, image rows on partitions) and the
W-direction conv is shifted slices of the moving operand accumulated
into PSUM.

The W-direction needs 4 shifted streams per output in the naive form.
Because the kernel's W-profile is symmetric ([k0,k1,k1,k0]), taps pair
up: px = x[w-2]+x[w+1] (weight k0) and qx = x[w-1]+x[w] (weight k1), so
a tile can instead stream px and qx (2 streams) with the pair weights
folded into the stationary band. The pairs are built on the vector
engine: one misaligned tensor_copy (2x_2P mode) makes a 1-elem-shifted
alias g2, then two 4B-aligned tensor_tensor adds (2x_1P mode) produce
px/qx (the direct adds have odd element offsets, which drop TT to 1x).
DVE cost is ~2x the PE savings per tile, but DVE is otherwise idle, so
a tuned fraction N_PQ of the 32 tile passes run the 2-stream path and
the rest run 4-shift, balancing PE (~79us) against DVE (~67us) under
the scalar engine's copy ceiling.

The steady-state pacer is the scalar (ACT) engine: every output element
crosses PSUM->SBUF through a 1x fp32 copy (~2.0us per 128x2048 pass) --
DMA cannot read PSUM and no engine copies fp32-from-PSUM faster. Hence:
one full-pass FD=2048 copy per pass (not two halves), all copies on ACT
(a PSUM copy on the DVE's strict FIFO freezes pair production for
multiple us), stores merged A+B per oct (one 1 MB DMA via a rearranged
dest AP). Pipeline depths (psum bufs=2x4banks, outp=4, xin=6, pqp=6)
are sized so PE never idles >~1us, keeping the HAM clock gate at
2.4 GHz; the H-edge remainder runs after the main loop because inserting
it mid-stream cost ~12us of pipeline disruption. ~38 junk warmup
matmuls bridge the clock gate through the initial DMA ramp.

Precision: harness gate is rel_err < 2e-2; plain fp16 (~7e-4) passes.

Layout: 8 planes packed per DRAM/SBUF row ("oct") with zero gaps (2
leading + 4 inter-plane) so every shifted slice is full 256 cols with W
zero-padding from the gaps. 4176 B rows. fp16 output (cast during the
PSUM->SBUF copy), 260-row-per-oct output (junk rows 127, 253..255).
Loads + weights on the sync HWDGE ring, stores on the scalar ring
(splitting stores across rings measurably hurts load latency).

Measured on 8 cores: HW exec ~117us (baseline hi/lo 4-shift: ~254us),
max rel err ~7e-4 vs the fp32 jax reference.
"""
import sys

for _p in ("/opt/trn_rl_repo", "/opt/pypackages"):
    if _p not in sys.path:
        sys.path.insert(0, _p)

import contextlib

import numpy as np


def _install_ntff_hook_shim():
    """The agent image's antenv lacks axon_hooks, which bass_utils needs
    for trace=True under axon. Provide it in sys.modules, backed by
    trn_agent_boot's ctypes NTFF shim."""
    import types

    if "antenv.axon_hooks" in sys.modules:
        return
    mod = types.ModuleType("antenv.axon_hooks")
    state = {"hook": None, "tried": False}

    def set_axon_ntff_profile_hook(hook):
        state["hook"] = hook

    def get_axon_ntff_profile_hook():
        if state["hook"] is None and not state["tried"]:
            state["tried"] = True
            try:
                from trn_agent_boot.trn_boot import _ntff_profile_via_ctypes

                state["hook"] = _ntff_profile_via_ctypes("/opt/axon/libaxon_pjrt.so")
            except Exception:
                state["hook"] = None
        return state["hook"]

    mod.set_axon_ntff_profile_hook = set_axon_ntff_profile_hook
    mod.get_axon_ntff_profile_hook = get_axon_ntff_profile_hook
    sys.modules["antenv.axon_hooks"] = mod
    try:
        import antenv

        antenv.axon_hooks = mod
    except ImportError:
        pass


_install_ntff_hook_shim()

import concourse.bacc as bacc
import concourse.tile as tile
from concourse import mybir
from concourse.bass_utils import run_bass_kernel_spmd

N_CORES = 8
H = W = 256
PLANES = 1024 // N_CORES  # 128 per core
OCT = 8  # planes packed per SBUF/DRAM row
NOCT = PLANES // OCT  # 16 oct-groups per core
STR = 260  # per-plane stride inside a packed row
GOFF = 2  # leading zero cols (W left pad)
GW = 2088  # packed row width in fp16 elems (4176 B); cols 2078.. are zero
OW = OCT * W  # 2048 out cols per packed row
PQW = 7 * STR + W  # 2076: width of the px/qx pair arrays

MA, MB = 127, 125  # out rows per A/B tile (A: x rows 0..128, B: 125..253)

N_WARM = 38  # junk matmuls bridging the HAM clock-gate through the DMA ramp
N_PQ = 21  # of the 32 A/B tile passes, how many use the 2-stream pq path
DVE_COPY = ()  # keep DVE free: any psum-copy on it freezes pair production


def _pq_ok(wk: np.ndarray) -> bool:
    return bool(
        np.allclose(wk[:, 0], wk[:, 3], rtol=1e-6)
        and np.allclose(wk[:, 1], wk[:, 2], rtol=1e-6)
    )


def _make_weights(wk: np.ndarray):
    """wk: flipped 4x4 kernel (wk[d, s] = weight for H-tap d, W-shift s).
    Shift-form stationaries wa/wb [128, 4*128], wr [96, 4*64]; pair-form
    wpa/wpb [128, 2*128] (stream 0: px with col-0 weights, stream 1: qx
    with col-1 weights)."""
    wa = np.zeros((128, 4, 128), np.float32)
    wpa = np.zeros((128, 2, 128), np.float32)
    for k in range(128):
        for m in range(MA):
            d = k - m + 2
            if 0 <= d <= 3:
                wa[k, :, m] = wk[d, :]
                wpa[k, 0, m] = wk[d, 0]
                wpa[k, 1, m] = wk[d, 1]
    wb = np.zeros((128, 4, 128), np.float32)
    wpb = np.zeros((128, 2, 128), np.float32)
    for k in range(128):
        for m in range(MB):
            d = k - m
            if 0 <= d <= 3:
                wb[k, :, m] = wk[d, :]
                wpb[k, 0, m] = wk[d, 0]
                wpb[k, 1, m] = wk[d, 1]
    wr = np.zeros((96, 4, 64), np.float32)
    for b in range(16):
        for k in range(6):
            for r in range(4):
                d = k - r
                if 0 <= d <= 3:
                    wr[6 * b + k, :, 4 * b + r] = wk[d, :]
    f16 = np.float16
    w = np.concatenate(
        [
            wa.reshape(128, 512),
            wb.reshape(128, 512),
            wpa.reshape(128, 256),
            wpb.reshape(128, 256),
        ],
        axis=1,
    )
    return w.astype(f16), wr.reshape(96, 256).astype(f16)


def _build_program(noct: int, n_pq: int):
    nc = bacc.Bacc("TRN2", target_bir_lowering=False, debug=False)
    f16, f32 = mybir.dt.float16, mybir.dt.float32

    d_xs = nc.dram_tensor("xs", [noct, H, GW], f16, kind="ExternalInput").ap()
    d_w = nc.dram_tensor("w", [128, 1536], f16, kind="ExternalInput").ap()
    d_wr = nc.dram_tensor("wr", [96, 256], f16, kind="ExternalInput").ap()
    d_out = nc.dram_tensor("out", [noct, H + 4, OW], f16, kind="ExternalOutput").ap()

    with tile.TileContext(nc) as tc, contextlib.ExitStack() as ctx:
        wpool = ctx.enter_context(tc.tile_pool(name="wpool", bufs=1))
        xin = ctx.enter_context(tc.tile_pool(name="xin", bufs=6))
        xinr = ctx.enter_context(tc.tile_pool(name="xinr", bufs=1))
        pqp = ctx.enter_context(tc.tile_pool(name="pqp", bufs=6))
        psum = ctx.enter_context(tc.tile_pool(name="psum", bufs=2, space="PSUM"))
        outp = ctx.enter_context(tc.tile_pool(name="outp", bufs=4))
        outr = ctx.enter_context(tc.tile_pool(name="outr", bufs=1))

        # all weights ride the sync ring in one DMA ahead of the first load
        t_w = wpool.tile([128, 1536], f16, tag="w")
        nc.sync.dma_start(out=t_w[:], in_=d_w)
        t_wa = t_w[:, 0:512]
        t_wb = t_w[:, 512:1024]
        t_wpa = t_w[:, 1024:1280]
        t_wpb = t_w[:, 1280:1536]

        # PE warmup: junk matmuls (no data deps) run during the DMA ramp and
        # hold the HAM clock gate at 2.4 GHz until the real stream starts.
        warm = wpool.tile([128, W], f16, tag="warm")
        nc.vector.memset(warm[:], 0.0)
        psW = psum.tile([128, W], f32, tag="ps")
        for _ in range(N_WARM):
            nc.tensor.matmul(
                psW[:, :], warm[:, :128], warm[:, :],
                start=True, stop=True, skip_group_check=True,
            )

        def shift_mms(ps, wt, xt, qbase):
            """4-shift path: 16 matmuls (4 shifts x 4 plane-windows)."""
            for s in range(4):
                lhsT = wt[:, s * 128 : s * 128 + 128]
                for qq in range(4):
                    q = qbase + qq
                    nc.tensor.matmul(
                        ps[:, qq * W : qq * W + W],
                        lhsT,
                        xt[:, STR * q + s : STR * q + s + W],
                        start=(s == 0 and qq % 2 == 0),
                        stop=(s == 3 and qq == 3),
                        skip_group_check=True,
                    )

        def pq_mms(ps, wt, px, qx, qbase):
            """pair path: 8 matmuls (2 streams x 4 plane-windows)."""
            for s in range(2):
                lhsT = wt[:, s * 128 : s * 128 + 128]
                src = px if s == 0 else qx
                for qq in range(4):
                    q = qbase + qq
                    nc.tensor.matmul(
                        ps[:, qq * W : qq * W + W],
                        lhsT,
                        src[:, STR * q : STR * q + W],
                        start=(s == 0 and qq % 2 == 0),
                        stop=(s == 1 and qq == 3),
                        skip_group_check=True,
                    )

        def do_pass(idx, xt, wt, wpt, o, ohalf):
            """One A/B tile pass: fill psum (2 halves) and copy into o's
            [ohalf] 2048-col block."""
            # Bresenham spread of pq vs shift; offset makes pass 0 a shift
            # pass so the first matmuls don't wait on the DVE pair chain
            pq = ((idx + 1) * n_pq) % 32 < n_pq
            if pq:
                # pair construction scheduled ~2 passes early (high_priority)
                # so DVE stays ahead of the PE's consumption
                with tc.high_priority(offset=70):
                    g2 = pqp.tile([128, GW], f16, tag="g2")
                    nc.vector.tensor_copy(g2[:, 0:2086], xt[:, 1:2087])
                    px = pqp.tile([128, PQW], f16, tag="px")
                    nc.vector.tensor_add(px[:, 0:PQW], xt[:, 0:PQW], g2[:, 2 : 2 + PQW])
                    qx = pqp.tile([128, PQW], f16, tag="qx")
                    nc.vector.tensor_add(qx[:, 0:PQW], g2[:, 0:PQW], xt[:, 2 : 2 + PQW])
            ps = psum.tile([128, 2048], f32, tag="ps")
            for half in (0, 1):
                if pq:
                    pq_mms(ps[:, half * 1024 : half * 1024 + 1024], wpt, px, qx, 4 * half)
                else:
                    shift_mms(ps[:, half * 1024 : half * 1024 + 1024], wt, xt, 4 * half)
            # one full-pass copy halves the scalar engine's per-op overhead
            dst = o[:, ohalf * 2048 : ohalf * 2048 + 2048]
            nc.scalar.copy(dst, ps[:, :])

        for g in range(noct):
            ta = xin.tile([128, GW], f16, tag="ta")
            if g == 0:
                # split the very first load so the PE stream starts sooner
                nc.sync.dma_start(out=ta[0:64, :], in_=d_xs[g, 0:64, :])
                nc.sync.dma_start(out=ta[64:128, :], in_=d_xs[g, 64:128, :])
            else:
                nc.sync.dma_start(out=ta[:], in_=d_xs[g, 0:128, :])
            tb = xin.tile([128, GW], f16, tag="tb")
            nc.sync.dma_start(out=tb[:], in_=d_xs[g, 125:253, :])

            oab = outp.tile([128, 2 * OW], f16, tag="oab")
            do_pass(2 * g, ta, t_wa, t_wpa, oab, 0)
            do_pass(2 * g + 1, tb, t_wb, t_wpb, oab, 1)
            dst = d_out[g, 0:256, :].rearrange("(two p) w -> p two w", two=2)
            src = oab[:, :].rearrange("p (two w) -> p two w", two=2)
            nc.scalar.dma_start(out=dst, in_=src)

            if g == 3:
                # prefetch the remainder inputs; the compute runs after the
                # main loop so it can't perturb the steady-state pipeline
                t_wr = wpool.tile([96, 256], f16, tag="wr")
                nc.sync.dma_start(out=t_wr[:], in_=d_wr)
                tr = xinr.tile([96, GW], f16, tag="tr")
                nc.sync.dma_start(out=tr[: 6 * noct, :], in_=d_xs[0:noct, 250:256, :])

        # stacked remainder for all octs: out rows 252..255 from x rows
        # 250..255 (bottom zero pad handled by the band); final copies go
        # on the vector engine, which has no pair work left by now
        orr = outr.tile([64, OW], f16, tag="orr")
        for half in (0, 1):
            psR = psum.tile([64, 1024], f32, tag="ps")
            for s in range(4):
                lhsT = t_wr[: 6 * noct, s * 64 : s * 64 + 4 * noct]
                for qq in range(4):
                    q = 4 * half + qq
                    nc.tensor.matmul(
                        psR[: 4 * noct, qq * W : qq * W + W],
                        lhsT,
                        tr[: 6 * noct, STR * q + s : STR * q + s + W],
                        start=(s == 0 and qq % 2 == 0),
                        stop=(s == 3 and qq == 3),
                        skip_group_check=True,
                    )
            nc.vector.tensor_copy(
                orr[: 4 * noct, half * 1024 : half * 1024 + 1024],
                psR[: 4 * noct, :],
            )
        nc.scalar.dma_start(out=d_out[0:noct, H : H + 4, :], in_=orr[: 4 * noct, :])

    nc.compile()
    return nc


_CACHE = {}


def _get_program(noct: int, n_pq: int):
    key = (noct, n_pq)
    if key not in _CACHE:
        _CACHE[key] = _build_program(noct, n_pq)
    return _CACHE[key]


def _run(x: np.ndarray, wk: np.ndarray, trace: bool = False):
    """x: [P, 256, 256] f32 full stack of planes (P divisible by 8*OCT),
    wk: flipped 4x4 kernel. Returns ([P, 256, 256] f32, exec_time_ns|None)."""
    P = x.shape[0]
    noct = P // (N_CORES * OCT)
    n_pq = N_PQ if _pq_ok(wk) else 0
    x16 = x.astype(np.float16).reshape(N_CORES, noct, OCT, H, W)
    xs = np.zeros((N_CORES, noct, H, GW), np.float16)
    for q in range(OCT):
        xs[:, :, :, GOFF + STR * q : GOFF + STR * q + W] = x16[:, :, q]

    w, wr = _make_weights(wk)
    nc = _get_program(noct, n_pq)

    in_maps = [
        {"xs": np.ascontiguousarray(xs[c]), "w": w, "wr": wr}
        for c in range(N_CORES)
    ]
    res = run_bass_kernel_spmd(nc, in_maps, list(range(N_CORES)), trace=trace)
    outq = np.stack([r["out"] for r in res.results])  # [8, noct, 260, 2048]
    outq = np.concatenate(
        [outq[:, :, 0:127], outq[:, :, 128:253], outq[:, :, 256:260]], axis=2
    )  # drop junk rows -> [8, noct, 256, 2048]
    out = np.empty((N_CORES, noct, OCT, H, W), np.float32)
    for q in range(OCT):
        out[:, :, q] = outq[:, :, :, W * q : W * q + W].astype(np.float32)
    return np.ascontiguousarray(out.reshape(P, H, W)), res.exec_time_ns


def kernel(input: np.ndarray, kernel: np.ndarray) -> np.ndarray:
    x = np.asarray(input, dtype=np.float32)
    k = np.asarray(kernel, dtype=np.float32)
    n, c, h, w = x.shape
    wk = np.flip(k, (0, 1)).copy()  # correlation weights
    out, _ = _run(x.reshape(n * c, h, w), wk, trace=False)
    return out.reshape(n, c, h, w)
